# revision 16
# baseline (speedup 1.0000x reference)
"""Trainium2 Bass kernel for 12-head MHA (B=2, S=4096, D=768), fp32.

Sharding: 8 cores = 2 batches x 4 head-groups (3 heads each).
Per core: Q/K/V projections, scores^T = K @ Q^T, exp (ScalarE, fused
1/8 scale), AV with a ones column appended to V (softmax denominator
lands in PSUM row 64 for free), normalize, and a partial
out-projection context @ Wo_slice^T in bf16. Host sums the 4 partial
outputs per batch and adds bo.

v4 structure (per-core span ~0.5ms, ScalarE exp-bound):
  - xT resident in SBUF (4 big DMAs); packed weights, one DMA.
  - head-major weight packing [q0 k0 | q1 k1 | q2 k2 | v0 v1 v2].
  - K^T packed even/odd key blocks on partition halves (strided PSUM
    copies); Q^T duplicated on both halves so QK^T row-pairs two K=64
    matmuls in the PE array.
  - exp groups of 3 key blocks (FD=1536) to amortize ACT overhead.
  - softmax normalize via DRAM-bounce partition broadcast.
  - out-projection of tile t-1 interleaved behind attention of tile t
    (PSUM slots shared with the AV accumulator tag); bf16 output,
    one batched store per 512 rows.
"""

import numpy as np

B, S, D = 2, 4096, 768
H, DK = 12, 64
NCORES = 8
HPC = 3                 # heads per core
DCH = D // 128          # 6 contraction chunks of 128
NT = S // 512           # 8 q-tiles / key-windows of 512
NKB = S // 128          # 32 key blocks of 128
GSZ = 2                 # key blocks per exp group

# probe knobs (harness uses defaults)
_BUILD_R = 1
_BUILD_BARRIER = False

_CACHE = {}


def _build_bass():
    from contextlib import ExitStack

    import concourse.bass as bass  # noqa: F401
    import concourse.mybir as mybir
    import concourse.tile as tile
    from concourse import bacc

    f32 = mybir.dt.float32
    bf16 = mybir.dt.bfloat16
    Exp = mybir.ActivationFunctionType.Exp

    nc = bacc.Bacc("TRN2", target_bir_lowering=False, debug=False)

    def mm(out, lhsT, rhs, **kw):
        nc.tensor.matmul(out, lhsT=lhsT, rhs=rhs, **kw)

    xT = nc.declare_dram_parameter("xT", [D, S], bf16, isOutput=False)
    # packed [q0 k0 q1 k1 q2 k2 v0 v1 v2] columns (64 each)
    wpk = nc.declare_dram_parameter("wpk", [D, 9 * DK], bf16, isOutput=False)
    bpk = nc.declare_dram_parameter("bpk", [1, 9 * DK], bf16, isOutput=False)
    woT = nc.declare_dram_parameter("woT", [HPC * DK, D], bf16, isOutput=False)
    out = nc.declare_dram_parameter("out", [S, D], bf16, isOutput=True)

    with tile.TileContext(nc) as tc, ExitStack() as ctx:
        const = ctx.enter_context(tc.tile_pool(name="const", bufs=1))
        pdata = ctx.enter_context(tc.tile_pool(name="pdata", bufs=1))

        ones = const.tile([1, 512], bf16, name="ones")
        nc.vector.memset(ones, 1.0)

        w_sb = const.tile([128, DCH, 9 * DK], bf16, name="w_sb")
        b_sb = const.tile([1, 9 * DK], bf16, name="b_sb")
        wo_a = const.tile([128, D], bf16, name="wo_a")
        wo_b = const.tile([64, D], bf16, name="wo_b")
        xsb = [
            const.tile([128, DCH, 1024], bf16, name=f"xsb{q}") for q in range(4)
        ]
        nc.sync.dma_start(out=w_sb, in_=wpk.rearrange("(c p) m -> p c m", p=128))
        nc.sync.dma_start(out=b_sb, in_=bpk[:, :])
        for q in range(4):
            qs = slice(q * 1024, (q + 1) * 1024)
            nc.sync.dma_start(
                out=xsb[q],
                in_=xT[:, qs].rearrange("(c p) n -> p c n", p=128),
            )
        nc.sync.dma_start(out=wo_a, in_=woT[0:128, :])
        nc.sync.dma_start(out=wo_b, in_=woT[128:192, :])

        qdup = [
            [
                pdata.tile([128, 512], bf16, name=f"qd{h}_{t}", tag=f"qd{h}_{t}")
                for t in range(NT)
            ]
            for h in range(HPC)
        ]
        kt = [
            pdata.tile([128, NKB * 64], bf16, name=f"kt{h}", tag=f"kt{h}")
            for h in range(HPC)
        ]
        vaug = [
            pdata.tile([128, NKB, 65], bf16, name=f"va{h}", tag=f"va{h}")
            for h in range(HPC)
        ]
        ctxA = [
            pdata.tile([128, 512], bf16, name=f"ctxA{t}", tag=f"ctxA{t}")
            for t in range(NT)
        ]
        ctxB = [
            pdata.tile([64, 512], bf16, name=f"ctxB{t}", tag=f"ctxB{t}")
            for t in range(NT)
        ]

        for h in range(HPC):
            nc.vector.memset(vaug[h][:, :, 64:65], 1.0)

        for r in range(_BUILD_R):
            # ---- Phase 0: project q0/k0 and all V (separate PSUM pool) ----
            with tc.tile_pool(name=f"ph0p_{r}", bufs=1, space="PSUM") as ph0p:
                def project_qk(h, pool, tag):
                    c0 = 2 * h * DK
                    for w in range(NT):
                        xq = xsb[w // 2]
                        wsl = slice((w % 2) * 512, (w % 2) * 512 + 512)
                        pg = pool.tile(
                            [128, 512], f32, name=f"pg{h}_{w}_{r}", tag=tag,
                            bufs=2,
                        )
                        for c in range(DCH):
                            mm(pg, lhsT=w_sb[:, c, c0 : c0 + 128],
                               rhs=xq[:, c, wsl], start=(c == 0), stop=False)
                        mm(pg, lhsT=b_sb[:, c0 : c0 + 128], rhs=ones[:, :],
                           start=False, stop=True)
                        nc.vector.tensor_copy(qdup[h][w][0:64, :], pg[0:64, :])
                        nc.vector.tensor_copy(qdup[h][w][64:128, :], pg[0:64, :])
                        wcols = slice(w * 256, (w + 1) * 256)
                        src = pg[64:128, :].rearrange(
                            "p (b lo n) -> p lo b n", lo=2, n=128
                        )
                        dst = kt[h][:, wcols].rearrange("p (b n) -> p b n", n=128)
                        nc.vector.tensor_copy(dst[0:64, :, :], src[:, 0, :, :])
                        nc.vector.tensor_copy(dst[64:128, :, :], src[:, 1, :, :])

                project_qk(0, ph0p, "pg0")
                for w in range(NT):
                    xq = xsb[w // 2]
                    for sc in range(4):
                        j = w * 4 + sc
                        ksl = slice((j % 8) * 128, (j % 8) * 128 + 128)
                        pv = ph0p.tile(
                            [128, HPC * DK], f32, name=f"pv{w}_{sc}_{r}",
                            tag="pv", bufs=2,
                        )
                        for c in range(DCH):
                            mm(pv, lhsT=xq[:, c, ksl], rhs=w_sb[:, c, 384:576],
                               start=(c == 0), stop=False)
                        mm(pv, lhsT=ones[:, 0:128], rhs=b_sb[:, 384:576],
                           start=False, stop=True)
                        for h in range(HPC):
                            nc.vector.tensor_copy(
                                vaug[h][:, j, 0:64], pv[:, h * DK : (h + 1) * DK]
                            )

            # ---- Phase 2: attention; heads 1-2 project in parallel ----
            with (
                tc.tile_pool(name=f"ph2_{r}", bufs=1) as ph2,
                tc.tile_pool(name=f"ph2p_{r}", bufs=1, space="PSUM") as ph2p,
                tc.tile_pool(name=f"rcdp_{r}", bufs=2, space="DRAM") as rcdp,
            ):
                def attention_tile(t, h):
                    pav = ph2p.tile(
                        [65, 512], f32, name=f"av{t}_{h}_{r}", tag="acc",
                        bufs=2,
                    )

                    def av_mms(blocks, et):
                        for i, j in enumerate(blocks):
                            mm(
                                pav,
                                lhsT=vaug[h][:, j, :],
                                rhs=et[:, i * 512 : (i + 1) * 512],
                                start=(j == 0), stop=(j == NKB - 1),
                            )

                    # AV of group g is emitted after the scores of group g+1,
                    # so the PE never queue-blocks on the exp of the group it
                    # just fed (cross-engine sem is satisfied by the time the
                    # PE reaches the AV matmuls).
                    pending = None
                    for g0 in range(0, NKB, GSZ):
                        blocks = list(range(g0, min(g0 + GSZ, NKB)))
                        nb = len(blocks)
                        ps = ph2p.tile(
                            [128, GSZ * 512], f32,
                            name=f"sc{t}_{h}_{g0}_{r}", tag="scores", bufs=2,
                        )
                        for i, j in enumerate(blocks):
                            pb = (j % 2) * 64
                            col0 = (j // 4) * 256 + ((j % 4) // 2) * 128
                            mm(
                                ps[:, i * 512 : (i + 1) * 512],
                                lhsT=kt[h][pb : pb + 64, col0 : col0 + 128],
                                rhs=qdup[h][t][pb : pb + 64, :],
                                start=True, stop=True,
                            )
                        et = ph2.tile(
                            [128, GSZ * 512], bf16,
                            name=f"et{t}_{h}_{g0}_{r}", tag="et", bufs=4,
                        )
                        nc.scalar.activation(
                            et[:, 0 : nb * 512], ps[:, 0 : nb * 512], Exp,
                            scale=0.125,
                        )
                        if pending is not None:
                            av_mms(*pending)
                        pending = (blocks, et)
                    av_mms(*pending)
                    rc = ph2.tile(
                        [65, 512], f32, name=f"rc{t}_{h}_{r}", tag="rc",
                        bufs=2,
                    )
                    nc.vector.reciprocal(rc[64:65, :], pav[64:65, :])
                    rcd = rcdp.tile(
                        [1, 512], f32, name=f"rcd{t}_{h}_{r}", tag="rcd"
                    )
                    nc.sync.dma_start(out=rcd, in_=rc[64:65, :])
                    bc = ph2.tile(
                        [64, 512], f32, name=f"bc{t}_{h}_{r}", tag="bc",
                        bufs=2,
                    )
                    nc.sync.dma_start(out=bc, in_=rcd.partition_broadcast(64))
                    if h == 0:
                        dst = ctxA[t][0:64, :]
                    elif h == 1:
                        dst = ctxA[t][64:128, :]
                    else:
                        dst = ctxB[t][0:64, :]
                    nc.vector.tensor_mul(dst, pav[0:64, :], bc)

                def outproj(t):
                    ot = ph2.tile(
                        [128, 4, D], bf16, name=f"ot{t}_{r}", tag="ot", bufs=2
                    )
                    for sci in range(4):
                        csl = slice(sci * 128, (sci + 1) * 128)
                        po1 = ph2p.tile(
                            [128, 512], f32, name=f"po1_{t}_{sci}_{r}",
                            tag="acc", bufs=2,
                        )
                        po2 = ph2p.tile(
                            [128, 256], f32, name=f"po2_{t}_{sci}_{r}",
                            tag="acc", bufs=2,
                        )
                        mm(po1, lhsT=ctxA[t][:, csl], rhs=wo_a[:, 0:512],
                           start=True, stop=False)
                        mm(po1, lhsT=ctxB[t][:, csl], rhs=wo_b[:, 0:512],
                           start=False, stop=True)
                        mm(po2, lhsT=ctxA[t][:, csl], rhs=wo_a[:, 512:768],
                           start=True, stop=False)
                        mm(po2, lhsT=ctxB[t][:, csl], rhs=wo_b[:, 512:768],
                           start=False, stop=True)
                        nc.vector.tensor_copy(ot[:, sci, 0:512], po1)
                        nc.vector.tensor_copy(ot[:, sci, 512:768], po2)
                    nc.sync.dma_start(
                        out=out[t * 512 : (t + 1) * 512, :].rearrange(
                            "(j rr) d -> rr j d", rr=128
                        ),
                        in_=ot,
                    )

                # head-0 attention while heads 1-2 project into the pg tag
                for t in range(NT):
                    attention_tile(t, 0)
                project_qk(1, ph2p, "pg")
                project_qk(2, ph2p, "pg")
                for t in range(NT):
                    attention_tile(t, 1)
                    attention_tile(t, 2)
                    if t > 0:
                        outproj(t - 1)
                outproj(NT - 1)

            if _BUILD_BARRIER and r < _BUILD_R - 1:
                tc.strict_bb_all_engine_barrier()

    nc.compile()
    return nc


def _get_nc():
    if "nc" not in _CACHE:
        _CACHE["nc"] = _build_bass()
    return _CACHE["nc"]


def make_in_maps(x, Wq, bq, Wk, bk, Wv, bv, Wo, bo):
    """Per-core input dicts (host-side sharding + layout prep, bf16 cast)."""
    import ml_dtypes

    bf = ml_dtypes.bfloat16
    x = np.asarray(x, dtype=np.float32)
    Wq, Wk, Wv, Wo = (np.asarray(a, np.float32) for a in (Wq, Wk, Wv, Wo))
    bq, bk, bv = (np.asarray(a, np.float32) for a in (bq, bk, bv))
    in_maps = []
    for c in range(NCORES):
        b = c // 4
        h0 = (c % 4) * HPC
        cols = []
        bcols = []
        for h in range(HPC):
            hs = slice((h0 + h) * DK, (h0 + h + 1) * DK)
            cols += [Wq[hs].T, Wk[hs].T]
            bcols += [bq[hs], bk[hs]]
        rows = slice(h0 * DK, (h0 + HPC) * DK)
        cols.append(Wv[rows].T)
        bcols.append(bv[rows])
        wpk = np.concatenate(cols, axis=1)
        bpk = np.concatenate(bcols)[None, :]
        in_maps.append(
            {
                "xT": np.ascontiguousarray(x[b].T).astype(bf),
                "wpk": np.ascontiguousarray(wpk).astype(bf),
                "bpk": np.ascontiguousarray(bpk).astype(bf),
                "woT": np.ascontiguousarray(Wo[:, rows].T).astype(bf),
            }
        )
    return in_maps


def kernel(x, Wq, bq, Wk, bk, Wv, bv, Wo, bo, _trace=False):
    from concourse.bass_utils import run_bass_kernel_spmd

    nc = _get_nc()
    in_maps = make_in_maps(x, Wq, bq, Wk, bk, Wv, bv, Wo, bo)
    res = run_bass_kernel_spmd(
        nc, in_maps, core_ids=list(range(NCORES)), trace=_trace
    )
    _CACHE["last_results"] = res
    out = np.zeros((B, S, D), dtype=np.float32)
    for c in range(NCORES):
        out[c // 4] += np.asarray(res.results[c]["out"], dtype=np.float32)
    out += np.asarray(bo, dtype=np.float32)[None, None, :]
    return out
